# revision 1
# baseline (speedup 1.0000x reference)
"""FISTA compressed-sensing kernel for Trainium2 (8 NeuronCores, SPMD).

Problem: for each of 64 patches (x3 channels), run 200 FISTA iterations of
    min_x 0.5||A x - b||^2 + lam||x||_1,   A: (81, 5184)
Sharding: pure data-parallel over the batch — 8 patches x 3 channels = 24
columns per core; A replicated.

Per-core formulation (column matrix Y: (5184, 24)):
    Ay   = A @ Y                      (81, 24)
    G    = A^T @ Ay - Atb             (5184, 24)   [Atb folded into the
                                                    matmul via 24 extra
                                                    contraction rows]
    Z    = Y - mu*G
    Xn   = soft_threshold(Z, lam*mu)
    Y'   = Xn + coef_i * (Xn - X)

Layout: D=5184 padded to 5248 = 41*128; state tiles [128, kt, 24] with
d = kt*128 + p. Column groups g0 = ktiles 0..20, g1 = 21..40 so each
gradient group fits one PSUM bank.
"""

import os

import numpy as np

import concourse.bass as bass
import concourse.mybir as mybir
import concourse.tile as tile
from concourse.bass_utils import run_bass_kernel_spmd

F32 = mybir.dt.float32

M = 81            # measurements (9x9 camera patch)
D = 5184          # atoms (72x72 upsampled grid)
KT = 41           # 128-row tiles covering D (padded to 5248)
DP = KT * 128     # 5248
NCORES = 8
B = 64
BPC = B // NCORES           # 8 patches per core
N = BPC * 3                 # 24 state columns per core
ITERS = int(os.environ.get("FISTA_ITERS", "200"))
KA = 128                    # augmented contraction dim for matmul2
CT0 = 96                    # partition row where the -Atb^T block starts (32-aligned)
G0, G1 = 21, 20             # ktiles per column group (504 / 480 psum cols)

_CACHE = {}


def _legalize_waits(nc):
    """This walrus build accepts at most ONE semaphore wait per instruction
    (setupSyncWait: 'Too many sync wait commands'). Tile emits multi-wait
    instructions; split the excess waits onto injected same-engine NoOps
    placed immediately before the instruction (engine queues are FIFO, so
    semantics are identical)."""
    n = 0
    for fn in nc.m.functions:
        for bb in fn.blocks:
            insts = bb.instructions
            out = []
            changed = False
            for ins in insts:
                si = ins.sync_info
                ow = list(si.on_wait) if si is not None else []
                if len(ow) > 1 and ins.engine is not None:
                    for w in ow[:-1]:
                        n += 1
                        out.append(mybir.InstNoOp(
                            name=f"I-waitnop-{n}",
                            engine=ins.engine,
                            ins=[],
                            outs=[],
                            debug=ins.debug,
                            sync_info=mybir.SyncInfo(on_wait=[w], on_update=[]),
                        ))
                    ins.sync_info = mybir.SyncInfo(
                        on_wait=[ow[-1]], on_update=list(si.on_update))
                    changed = True
                out.append(ins)
            if changed:
                bb.instructions = out
    return n


def _fista_coefs(iters):
    t = 1.0
    coefs = []
    for _ in range(iters):
        t_new = (1.0 + float(np.sqrt(1.0 + 4.0 * t * t))) / 2.0
        coefs.append((t - 1.0) / t_new)
        t = t_new
    return coefs


def _build(mu_s, thr, iters):
    """Build the Bass module (same program for all 8 cores)."""
    nc = bass.Bass()
    coefs = _fista_coefs(iters)

    # DRAM parameters (per core): A^T tiles, A tiles, measurement matrix b.
    at_d = nc.declare_dram_parameter("at", [128, KT, M], F32, isOutput=False)
    a_d = nc.declare_dram_parameter("a", [128, KT, 128], F32, isOutput=False)
    b_d = nc.declare_dram_parameter("b", [M, N], F32, isOutput=False)
    ay0_d = nc.declare_dram_parameter("ayinit", [128, N], F32, isOutput=False)
    xout_d = nc.declare_dram_parameter("xout", [128, KT, N], F32, isOutput=True)

    with tile.TileContext(nc) as tc:
        with (
            tc.tile_pool(name="weights", bufs=1) as wpool,
            tc.tile_pool(name="state", bufs=1) as spool,
            tc.tile_pool(name="tmp", bufs=2) as tpool,
            tc.tile_pool(name="psum_ay", bufs=2, space="PSUM") as ppool_ay,
            tc.tile_pool(name="psum_gp", bufs=2, space="PSUM") as ppool_gp,
            tc.tile_pool(name="psum_ct", bufs=2, space="PSUM") as ppool_ct,
        ):
            # --- persistent SBUF tensors -------------------------------
            at_sb = wpool.tile([128, KT, M], F32)      # lhsT for matmul1
            w2_sb = wpool.tile([KA, KT, 128], F32)     # lhsT for matmul2
            ay_sb = wpool.tile([KA, N], F32)           # rhs for matmul2
            y_sb = [spool.tile([128, G0, N], F32, tag="y0", name="y0"),
                    spool.tile([128, G1, N], F32, tag="y1", name="y1")]
            # x ping-pong: x_sb[s][g]
            x_sb = [[spool.tile([128, G0, N], F32, tag=f"x{s}0", name=f"x{s}0"),
                     spool.tile([128, G1, N], F32, tag=f"x{s}1", name=f"x{s}1")]
                    for s in range(2)]

            nc.sync.dma_start(out=at_sb[:], in_=at_d[:])
            nc.sync.dma_start(out=w2_sb[:], in_=a_d[:])

            b_sb = wpool.tile([M, N], F32)
            nc.sync.dma_start(out=b_sb[:], in_=b_d[:])

            negthr = wpool.tile([128, 1], F32)
            nc.vector.memset(negthr[:], -thr)

            # rhs init: zeros with identity block at rows CT0..CT0+N
            nc.sync.dma_start(out=ay_sb[:], in_=ay0_d[:])

            # initial state: x = y = 0
            for g in range(2):
                nc.vector.memset(y_sb[g][:], 0.0)
                nc.vector.memset(x_sb[1][g][:], 0.0)

            # --- fold -Atb^T into rows CT0..CT0+N of w2 ----------------
            # ct = b^T A  (24, 5248), computed in chunks of 4 ktiles (512),
            # staged in SBUF then written into w2 with ONE instruction so
            # downstream LDWEIGHTS carry few sync waits.
            ct_stage = wpool.tile([N, KT * 128], F32)
            for c4 in range((KT + 3) // 4):
                k0 = c4 * 4
                nk = min(4, KT - k0)
                ct_ps = ppool_ct.tile([N, 4 * 128], F32, tag="ct")
                nc.tensor.matmul(
                    ct_ps[:, : nk * 128],
                    b_sb[:],
                    w2_sb[0:M, k0 : k0 + nk, :].rearrange("k a b -> k (a b)"),
                    start=True,
                    stop=True,
                )
                nc.vector.tensor_scalar_mul(
                    ct_stage[:, k0 * 128 : (k0 + nk) * 128],
                    ct_ps[:, : nk * 128],
                    -1.0,
                )
            nc.vector.tensor_copy(
                w2_sb[CT0 : CT0 + N, :, :].rearrange("k a b -> k (a b)"),
                ct_stage[:],
            )

            # group g -> (ktile offset, ktile count)
            gidx = [(0, G0), (G0, G1)]

            # --- FISTA iterations --------------------------------------
            for i in range(iters):
                cur, prev = i % 2, (i + 1) % 2

                # matmul1: Ay = A @ Y  -> psum (81, 24)
                ay_ps = ppool_ay.tile([M, N], F32, tag="ay")
                for kt in range(KT):
                    g, j = (0, kt) if kt < G0 else (1, kt - G0)
                    nc.tensor.matmul(
                        ay_ps[:],
                        at_sb[:, kt, :],
                        y_sb[g][:, j, :],
                        start=(kt == 0),
                        stop=(kt == KT - 1),
                    )
                nc.vector.tensor_copy(ay_sb[0:M, :], ay_ps[:])

                # matmul2 + elementwise, per column group
                for g in range(2):
                    k0, ng = gidx[g]
                    gp = ppool_gp.tile([128, ng, N], F32, tag=f"gp{g}")
                    for j in range(ng):
                        nc.tensor.matmul(
                            gp[:, j, :],
                            w2_sb[:, k0 + j, :],
                            ay_sb[:],
                            start=True,
                            stop=True,
                        )
                    # z = y - mu*(A^T Ay - Atb) = (gp * -mu) + y
                    z = tpool.tile([128, ng, N], F32, tag=f"z{g}")
                    nc.vector.scalar_tensor_tensor(
                        out=z[:],
                        in0=gp[:],
                        scalar=-mu_s,
                        in1=y_sb[g][:],
                        op0=mybir.AluOpType.mult,
                        op1=mybir.AluOpType.add,
                    )
                    # soft threshold: xn = relu(z - thr) - relu(-z - thr)
                    p = tpool.tile([128, ng, N], F32, tag=f"p{g}")
                    q = tpool.tile([128, ng, N], F32, tag=f"q{g}")
                    nc.scalar.activation(
                        out=p[:], in_=z[:],
                        func=mybir.ActivationFunctionType.Relu,
                        bias=negthr[:], scale=1.0,
                    )
                    nc.scalar.activation(
                        out=q[:], in_=z[:],
                        func=mybir.ActivationFunctionType.Relu,
                        bias=negthr[:], scale=-1.0,
                    )
                    xn = x_sb[cur][g]
                    nc.vector.tensor_sub(xn[:], p[:], q[:])
                    if i == 0:
                        # coef_0 = 0 -> y = xn
                        nc.gpsimd.tensor_copy(y_sb[g][:], xn[:])
                    else:
                        d = tpool.tile([128, ng, N], F32, tag=f"d{g}")
                        nc.gpsimd.tensor_sub(d[:], xn[:], x_sb[prev][g][:])
                        nc.vector.scalar_tensor_tensor(
                            out=y_sb[g][:],
                            in0=d[:],
                            scalar=coefs[i],
                            in1=xn[:],
                            op0=mybir.AluOpType.mult,
                            op1=mybir.AluOpType.add,
                        )

            # --- write back final x ------------------------------------
            fin = (iters - 1) % 2
            nc.sync.dma_start(out=xout_d[:, 0:G0, :], in_=x_sb[fin][0][:])
            nc.sync.dma_start(out=xout_d[:, G0:KT, :], in_=x_sb[fin][1][:])

    _legalize_waits(nc)
    return nc


def _prep_inputs(inp, A):
    """Host-side shard/reshape: returns per-core input maps."""
    A = np.asarray(A, np.float32)
    A_pad = np.zeros((M, DP), np.float32)
    A_pad[:, :D] = A
    a_tiles = np.zeros((128, KT, 128), np.float32)
    a_tiles[:M] = A_pad.reshape(M, KT, 128)
    ay_init = np.zeros((128, N), np.float32)
    ay_init[CT0 : CT0 + N] = np.eye(N, dtype=np.float32)
    at_tiles = np.ascontiguousarray(
        A_pad.T.reshape(KT, 128, M).transpose(1, 0, 2))  # [128, KT, M]

    inp = np.asarray(inp, np.float32)
    in_maps = []
    for c in range(NCORES):
        chunk = inp[c * BPC : (c + 1) * BPC]            # (8, 81, 3)
        b_mat = np.ascontiguousarray(chunk.transpose(1, 0, 2).reshape(M, N))
        in_maps.append({"at": at_tiles, "a": a_tiles, "b": b_mat,
                        "ayinit": ay_init})
    return in_maps


def _unshard(results):
    outs = []
    for c in range(NCORES):
        xo = np.asarray(results[c]["xout"])              # [128, KT, N]
        x_dn = xo.transpose(1, 0, 2).reshape(DP, N)[:D]  # (5184, 24)
        outs.append(x_dn.reshape(72, 72, BPC, 3).transpose(2, 0, 1, 3))
    return np.concatenate(outs, 0).astype(np.float32)    # (64, 72, 72, 3)


def _run(inp, A, lam, mu, trace=False):
    mu_s = float(np.asarray(mu).reshape(-1)[0])
    thr = float(np.asarray(lam).reshape(-1)[0]) * mu_s
    key = (mu_s, thr, ITERS)
    if key not in _CACHE:
        _CACHE[key] = _build(mu_s, thr, ITERS)
    nc = _CACHE[key]
    in_maps = _prep_inputs(inp, A)
    res = run_bass_kernel_spmd(nc, in_maps, list(range(NCORES)), trace=trace)
    return _unshard(res.results), res


def kernel(inp, A, lam, mu):
    out, _ = _run(inp, A, lam, mu)
    return out



# revision 4
# speedup vs baseline: 128.3620x; 128.3620x over previous
"""FISTA compressed-sensing kernel for Trainium2 (8 NeuronCores, SPMD).

Problem: for each of 64 patches (x3 channels), run 200 FISTA iterations of
    min_x 0.5||A x - b||^2 + lam||x||_1,   A: (81, 5184)
Sharding: pure data-parallel over the batch -- 8 patches x 3 channels = 24
columns per core; A replicated.

Per-core formulation (column matrix Y: (5184, 24)):
    Ay   = A @ Y                      (81, 24)
    G    = A^T @ Ay - Atb             (5184, 24)   [Atb folded into the
                                                    matmul via 24 extra
                                                    contraction rows; the
                                                    -Atb^T block is computed
                                                    host-side]
    Z    = Y - mu*G
    Xn   = soft_threshold(Z, lam*mu) = Z - clamp(Z, -thr, thr)
    Y'   = Xn + coef_i * (Xn - X)

Layout: D=5184 padded to 5248 = 41*128; state tiles [128, kt, 24] with
d = kt*128 + p. Column groups g0 = ktiles 0..20, g1 = 21..40 so each
gradient group fits one PSUM bank.

v2: the 200 iterations run in a hardware For_i loop (2 FISTA steps per
body so the x ping-pong is static; per-step momentum coefficient read
from an SBUF table indexed by the loop var). All math is fp32 -- the
LASSO fixed point is extremely sensitive to A/Atb perturbation, so
bf16/fp16 matmuls diverge from the reference trajectory (measured 0.9 /
0.24 rel err at 200 iters). The -Atb^T block is folded host-side into
24 extra contraction rows of the matmul2 weights.
"""

import os

import numpy as np

import concourse.bass as bass
import concourse.mybir as mybir
import concourse.tile as tile
from concourse.bass import ds
from concourse.bass_utils import run_bass_kernel_spmd

F32 = mybir.dt.float32
BF16 = mybir.dt.bfloat16

M = 81            # measurements (9x9 camera patch)
D = 5184          # atoms (72x72 upsampled grid)
KT = 41           # 128-row tiles covering D (padded to 5248)
DP = KT * 128     # 5248
NCORES = 8
B = 64
BPC = B // NCORES           # 8 patches per core
N = BPC * 3                 # 24 state columns per core
ITERS = int(os.environ.get("FISTA_ITERS", "200"))
CT0 = 96                    # partition row where the -Atb^T block starts
G0, G1 = 21, 20             # ktiles per column group (504 / 480 psum cols)

_CACHE = {}


def _legalize_waits(nc):
    """This walrus build accepts at most ONE semaphore wait per instruction
    (setupSyncWait: 'Too many sync wait commands'). Tile emits multi-wait
    instructions; split the excess waits onto injected same-engine NoOps
    placed immediately before the instruction (engine queues are FIFO, so
    semantics are identical)."""
    n = 0
    for fn in nc.m.functions:
        for bb in fn.blocks:
            insts = bb.instructions
            out = []
            changed = False
            for ins in insts:
                si = ins.sync_info
                ow = list(si.on_wait) if si is not None else []
                if len(ow) > 1 and ins.engine is not None:
                    for w in ow[:-1]:
                        n += 1
                        out.append(mybir.InstNoOp(
                            name=f"I-waitnop-{n}",
                            engine=ins.engine,
                            ins=[],
                            outs=[],
                            debug=ins.debug,
                            sync_info=mybir.SyncInfo(on_wait=[w], on_update=[]),
                        ))
                    ins.sync_info = mybir.SyncInfo(
                        on_wait=[ow[-1]], on_update=list(si.on_update))
                    changed = True
                out.append(ins)
            if changed:
                bb.instructions = out
    return n


def _fista_coefs(iters):
    t = 1.0
    coefs = []
    for _ in range(iters):
        t_new = (1.0 + float(np.sqrt(1.0 + 4.0 * t * t))) / 2.0
        coefs.append((t - 1.0) / t_new)
        t = t_new
    return coefs


def _build(mu_s, thr, iters):
    """Build the Bass module (same program for all 8 cores)."""
    assert iters % 2 == 0
    half = iters // 2
    nc = bass.Bass()

    # DRAM parameters (per core): A^T tiles, augmented-A tiles (with the
    # -Atb^T block folded host-side), ay identity init, coef table.
    at_d = nc.declare_dram_parameter("at", [128, KT, M], F32, isOutput=False)
    a_d = nc.declare_dram_parameter("a", [128, KT, 128], F32, isOutput=False)
    ay0_d = nc.declare_dram_parameter("ayinit", [128, N], F32, isOutput=False)
    cf_d = nc.declare_dram_parameter("coefs", [128, 2, half], F32,
                                     isOutput=False)
    xout_d = nc.declare_dram_parameter("xout", [128, KT, N], F32,
                                       isOutput=True)

    with tile.TileContext(nc) as tc:
        with (
            tc.tile_pool(name="weights", bufs=1) as wpool,
            tc.tile_pool(name="state", bufs=1) as spool,
            tc.tile_pool(name="tmp", bufs=2) as tpool,
            tc.tile_pool(name="psum_ay", bufs=2, space="PSUM") as ppool_ay,
            tc.tile_pool(name="psum_gp", bufs=2, space="PSUM") as ppool_gp,
        ):
            # --- persistent SBUF tensors -------------------------------
            at_sb = wpool.tile([128, KT, M], F32)     # lhsT for matmul1
            w2_sb = wpool.tile([128, KT, 128], F32)   # lhsT for matmul2
            ay_sb = wpool.tile([128, N], F32)         # rhs for matmul2
            cf_sb = wpool.tile([128, 2, half], F32)    # momentum coef table
            y_sb = [spool.tile([128, G0, N], F32, tag="y0", name="y0"),
                    spool.tile([128, G1, N], F32, tag="y1", name="y1")]
            # x ping-pong: x_sb[s][g]
            x_sb = [[spool.tile([128, G0, N], F32, tag=f"x{s}0", name=f"x{s}0"),
                     spool.tile([128, G1, N], F32, tag=f"x{s}1", name=f"x{s}1")]
                    for s in range(2)]

            nc.sync.dma_start(out=at_sb[:], in_=at_d[:])
            nc.sync.dma_start(out=w2_sb[:], in_=a_d[:])
            nc.sync.dma_start(out=ay_sb[:], in_=ay0_d[:])
            nc.sync.dma_start(out=cf_sb[:], in_=cf_d[:])

            # initial state: x_prev = y = 0
            for g in range(2):
                nc.vector.memset(y_sb[g][:], 0.0)
                nc.vector.memset(x_sb[1][g][:], 0.0)

            # group g -> (ktile offset, ktile count)
            gidx = [(0, G0), (G0, G1)]

            def fista_step(cur, prev, coef_ap):
                # matmul1: Ay = A @ Y  -> psum (81, 24)
                ay_ps = ppool_ay.tile([M, N], F32, tag="ay")
                for kt in range(KT):
                    g, j = (0, kt) if kt < G0 else (1, kt - G0)
                    nc.tensor.matmul(
                        ay_ps[:],
                        at_sb[:, kt, :],
                        y_sb[g][:, j, :],
                        start=(kt == 0),
                        stop=(kt == KT - 1),
                    )
                nc.vector.tensor_copy(ay_sb[0:M, :], ay_ps[:])

                # matmul2 + elementwise, per column group
                for g in range(2):
                    k0, ng = gidx[g]
                    gp = ppool_gp.tile([128, ng, N], F32, tag=f"gp{g}")
                    for j in range(ng):
                        nc.tensor.matmul(
                            gp[:, j, :],
                            w2_sb[:, k0 + j, :],
                            ay_sb[:],
                            start=True,
                            stop=True,
                        )
                    # z = y - mu*(A^T Ay - Atb) = (gp * -mu) + y
                    z = tpool.tile([128, ng, N], F32, tag=f"z{g}")
                    nc.vector.scalar_tensor_tensor(
                        out=z[:],
                        in0=gp[:],
                        scalar=-mu_s,
                        in1=y_sb[g][:],
                        op0=mybir.AluOpType.mult,
                        op1=mybir.AluOpType.add,
                    )
                    # soft threshold: xn = z - clamp(z, -thr, thr)
                    c = tpool.tile([128, ng, N], F32, tag=f"c{g}")
                    nc.vector.tensor_scalar(
                        out=c[:],
                        in0=z[:],
                        scalar1=thr,
                        scalar2=-thr,
                        op0=mybir.AluOpType.min,
                        op1=mybir.AluOpType.max,
                    )
                    xn = x_sb[cur][g]
                    nc.vector.tensor_sub(xn[:], z[:], c[:])
                    # momentum: y = xn + coef*(xn - x_prev)
                    d = tpool.tile([128, ng, N], F32, tag=f"d{g}")
                    nc.gpsimd.tensor_sub(d[:], xn[:], x_sb[prev][g][:])
                    nc.vector.scalar_tensor_tensor(
                        out=y_sb[g][:],
                        in0=d[:],
                        scalar=coef_ap,
                        in1=xn[:],
                        op0=mybir.AluOpType.mult,
                        op1=mybir.AluOpType.add,
                    )

            # --- FISTA iterations: hardware loop, 2 steps per body -----
            with tc.For_i(0, half, 1,
                          hint_engines=(mybir.EngineType.PE,)) as it:
                fista_step(0, 1, cf_sb[:, 0, ds(it, 1)])
                fista_step(1, 0, cf_sb[:, 1, ds(it, 1)])

            # --- write back final x ------------------------------------
            nc.sync.dma_start(out=xout_d[:, 0:G0, :], in_=x_sb[1][0][:])
            nc.sync.dma_start(out=xout_d[:, G0:KT, :], in_=x_sb[1][1][:])

    _legalize_waits(nc)
    return nc


def _prep_inputs(inp, A, iters=None):
    """Host-side shard/reshape: returns per-core input maps."""
    if iters is None:
        iters = ITERS
    A = np.asarray(A, np.float32)
    A_pad = np.zeros((M, DP), np.float32)
    A_pad[:, :D] = A
    base_tiles = np.zeros((128, KT, 128), np.float32)
    base_tiles[:M] = A_pad.reshape(M, KT, 128)
    ay_init = np.zeros((128, N), np.float32)
    ay_init[CT0 : CT0 + N] = np.eye(N, dtype=np.float32)
    at_tiles = np.ascontiguousarray(
        A_pad.T.reshape(KT, 128, M).transpose(1, 0, 2))

    coefs = _fista_coefs(iters)
    half = max(1, iters // 2)
    cf = np.zeros((2, half), np.float32)
    cf[0] = coefs[0::2]
    cf[1] = coefs[1::2]
    cf_tab = np.ascontiguousarray(
        np.broadcast_to(cf[None], (128, 2, half)).astype(np.float32))

    inp = np.asarray(inp, np.float32)
    in_maps = []
    for c in range(NCORES):
        chunk = inp[c * BPC : (c + 1) * BPC]            # (8, 81, 3)
        b_mat = chunk.transpose(1, 0, 2).reshape(M, N)  # (81, 24)
        ct = b_mat.T @ A_pad                            # (24, 5248) = (Atb)^T
        w2 = base_tiles.copy()
        w2[CT0 : CT0 + N] = -ct.reshape(N, KT, 128)
        in_maps.append({"at": at_tiles, "a": w2,
                        "ayinit": ay_init, "coefs": cf_tab})
    return in_maps


def _unshard(results):
    outs = []
    for c in range(NCORES):
        xo = np.asarray(results[c]["xout"])              # [128, KT, N]
        x_dn = xo.transpose(1, 0, 2).reshape(DP, N)[:D]  # (5184, 24)
        outs.append(x_dn.reshape(72, 72, BPC, 3).transpose(2, 0, 1, 3))
    return np.concatenate(outs, 0).astype(np.float32)    # (64, 72, 72, 3)


def _run(inp, A, lam, mu, trace=False):
    mu_s = float(np.asarray(mu).reshape(-1)[0])
    thr = float(np.asarray(lam).reshape(-1)[0]) * mu_s
    key = (mu_s, thr, ITERS)
    if key not in _CACHE:
        _CACHE[key] = _build(mu_s, thr, ITERS)
    nc = _CACHE[key]
    in_maps = _prep_inputs(inp, A)
    res = run_bass_kernel_spmd(nc, in_maps, list(range(NCORES)), trace=trace)
    return _unshard(res.results), res


def kernel(inp, A, lam, mu):
    out, _ = _run(inp, A, lam, mu)
    return out
